# revision 43
# baseline (speedup 1.0000x reference)
"""GCN message-passing kernel for Trainium2, 8-core data-parallel.

Reference computation (B=32, L=100, D=200, R=40 relations):
    embs = concat(emb[words], pos_emb[pos], ner_emb[ner])   [B,L,360]
    x = embs @ Wp.T + bp                                    [B,L,200]
    for (W,b) in layers:
        Ax[b,l,d] = sum_m deprel_emb[adj[b,l,m], d] * x[b,m,d]
        x = relu(((Ax + x) @ W.T + 2b) / denom)
    returns (x, mask)

Key identity used on device (exact):
    Ax = sum_{r=1..39} Ind_r @ (x * deprel[r])     Ind_r[m,l] = (adj[l,m]==r)
so the per-edge [B,L,L,D] gather never materializes; instead 39 masked
matmuls accumulate in PSUM per (batch, layer).

Sharding: batch axis across 8 cores (4 items each). Embedding tables are
constant-folded on the host (emb @ Wp_words.T; pos/ner projected into one
500-row table) and gathered on-device by index via indirect DMA.
"""

import sys

sys.path.insert(0, "/opt/trn_rl_repo")

import numpy as np
import ml_dtypes

BF16 = ml_dtypes.bfloat16

B, L, D = 32, 100, 200
R = 40  # deprel vocab (row 0 is padding -> dropped)
NR = R - 1  # 39 real relations
NCORES = 8
BLOC = B // NCORES  # 4 batch items per core

# "cb" consts layout (bf16, [100, CW]): waug | ident | onesc
OFF_WAUG = 0
OFF_IDENT = OFF_WAUG + 800        # 800
OFF_ONESC = OFF_IDENT + L         # 900
CW = OFF_ONESC + 1                # 901

_PROGRAM = None


def _build_program():
    import concourse.bass as bass
    import concourse.bacc as bacc
    import concourse.mybir as mybir
    from concourse.tile import TileContext
    from concourse.alu_op_type import AluOpType

    dt = mybir.dt
    nc = bacc.Bacc()

    # ---- DRAM parameters (per-core inputs) ----
    # adjTall: [L, BLOC*L] = all batches' transposed adj side by side
    # gall:    [L, BLOC*2] = all batches' gather indices side by side
    adjTall = nc.declare_dram_parameter(
        "adjTall", [L, BLOC * L], dt.bfloat16, isOutput=False
    )
    gall = nc.declare_dram_parameter("gall", [L, BLOC * 2], dt.int32, isOutput=False)
    gtab = nc.declare_dram_parameter("gtab", [50500, D], dt.bfloat16, isOutput=False)
    cb = nc.declare_dram_parameter("cb", [L, CW], dt.bfloat16, isOutput=False)
    ind0b = nc.declare_dram_parameter("ind0b", [L, NR * L], dt.bfloat16, isOutput=False)
    rpatb = nc.declare_dram_parameter("rpatb", [L, NR * L], dt.bfloat16, isOutput=False)
    depb = nc.declare_dram_parameter("depb", [L, NR * D], dt.bfloat16, isOutput=False)
    consts1 = nc.declare_dram_parameter("consts1", [1, 2 * D + L], dt.bfloat16, isOutput=False)

    xm_out = nc.declare_dram_parameter(
        "xm_out", [BLOC, L, D + 1], dt.float32, isOutput=True
    )

    AX = mybir.AxisListType.X
    Relu = mybir.ActivationFunctionType.Relu
    Copy = mybir.ActivationFunctionType.Copy

    # arrival-ordered: dep r 26..38 lands first, then 0..12, then 13..25
    Z_CHUNKS_FULL = ((26, 13), (0, 13), (13, 13))
    R_ORDER = list(range(26, 39)) + list(range(0, 26))

    with TileContext(nc) as tc:
        with (
            tc.tile_pool(name="consts", bufs=1) as cpool,
            tc.tile_pool(name="work", bufs=2) as wpool,
            tc.tile_pool(name="big", bufs=3) as bpool,
            tc.tile_pool(name="psA", bufs=2, space="PSUM") as psA,
            tc.tile_pool(name="psB", bufs=2, space="PSUM") as psB,
            tc.tile_pool(name="psC", bufs=2, space="PSUM") as psC,
        ):
            # ---- dual-queue load schedule, priority-ordered: tiny/hot loads
            # fully land before the fat deprel stream enters the rings ----
            ga_sb = wpool.tile([L, BLOC * 2], dt.int32, bufs=1)
            nc.sync.dma_start(out=ga_sb[:], in_=gall[:])
            atall_sb = wpool.tile([L, BLOC * L], dt.bfloat16, bufs=1)
            nc.scalar.dma_start(out=atall_sb[:], in_=adjTall[:])
            gi_t = [ga_sb[:, 2 * b : 2 * b + 2] for b in range(BLOC)]
            at_t = [atall_sb[:, b * L : (b + 1) * L] for b in range(BLOC)]

            c_sb = cpool.tile([L, CW], dt.bfloat16)
            nc.scalar.dma_start(out=c_sb[:], in_=cb[:])
            c1_sb = cpool.tile([1, 2 * D + L], dt.bfloat16)
            nc.scalar.dma_start(out=c1_sb[:], in_=consts1[:])

            dep_sb = cpool.tile([L, NR, D], dt.bfloat16)
            depb_v = depb[:].rearrange("p (r d) -> p r d", r=NR)
            ind_full = cpool.tile([L, BLOC, NR, L], dt.bfloat16)
            ind_t = [ind_full[:, b, :, :] for b in range(BLOC)]

            # fat streams in consumption order across the two queues:
            # b0's indicator arrives precomputed by DMA (critical path);
            # b1..b3 indicators are computed on DVE from rpat
            nc.scalar.dma_start(out=dep_sb[:, 26:39, :], in_=depb_v[:, 26:39, :])
            nc.sync.dma_start(
                out=ind_full[:, 0, :, :],
                in_=ind0b[:].rearrange("p (r l) -> p r l", r=NR),
            )
            nc.sync.dma_start(out=dep_sb[:, 0:13, :], in_=depb_v[:, 0:13, :])
            nc.scalar.dma_start(out=dep_sb[:, 13:26, :], in_=depb_v[:, 13:26, :])
            rpat_sb = cpool.tile([L, NR, L], dt.bfloat16)
            nc.sync.dma_start(
                out=rpat_sb[:], in_=rpatb[:].rearrange("p (r l) -> p r l", r=NR)
            )

            w_sb = c_sb[:, OFF_WAUG:OFF_IDENT]
            ident_sb = c_sb[:, OFF_IDENT:OFF_ONESC]
            onesc_sb = c_sb[:, OFF_ONESC : OFF_ONESC + 1]
            w2b_sb = c1_sb[:, 0 : 2 * D]
            onesr_sb = c1_sb[:, 2 * D : 2 * D + L]

            # ---- phase A per b: gathers, x0, denom/mask ----
            x_t = [None] * BLOC
            recip_t = [None] * BLOC
            for b in range(BLOC):
                wgpn_sb = wpool.tile([L, 2, D], dt.bfloat16)
                for j in range(2):
                    nc.gpsimd.indirect_dma_start(
                        out=wgpn_sb[:, j, :],
                        out_offset=None,
                        in_=gtab[:],
                        in_offset=bass.IndirectOffsetOnAxis(
                            ap=gi_t[b][:, j : j + 1], axis=0
                        ),
                    )
                x0_sb = wpool.tile([L, D], dt.bfloat16, tag="x", bufs=BLOC)
                nc.gpsimd.tensor_tensor(
                    out=x0_sb[:], in0=wgpn_sb[:, 0, :], in1=wgpn_sb[:, 1, :],
                    op=AluOpType.add,
                )
                x_t[b] = x0_sb

                # nz = min(adjT, 1): exact nonzero indicator for values 0..39
                nz_sb = wpool.tile([L, L], dt.bfloat16)
                nc.vector.tensor_scalar(
                    out=nz_sb[:], in0=at_t[b], scalar1=1.0, scalar2=None,
                    op0=AluOpType.min,
                )
                dn_ps = psC.tile([L, 1], dt.float32, tag="psC")
                nc.tensor.matmul(
                    out=dn_ps[:], lhsT=nz_sb[:], rhs=onesc_sb, start=True, stop=True
                )
                dnp1_sb = wpool.tile([L, 1], dt.float32)
                nc.scalar.add(out=dnp1_sb[:], in_=dn_ps[:], add=1.0)
                recip_sb = wpool.tile([L, 1], dt.float32, bufs=BLOC)
                nc.vector.reciprocal(out=recip_sb[:], in_=dnp1_sb[:])
                recip_t[b] = recip_sb

                msum_sb = wpool.tile([L, 1], dt.float32)
                nc.vector.tensor_reduce(
                    out=msum_sb[:], in_=nz_sb[:], axis=AX, op=AluOpType.add
                )
                tsum_sb = wpool.tile([L, 1], dt.float32)
                nc.vector.tensor_tensor(
                    out=tsum_sb[:], in0=msum_sb[:], in1=dn_ps[:], op=AluOpType.add
                )
                maskv_sb = wpool.tile([L, 1], dt.float32, bufs=BLOC)
                nc.vector.tensor_scalar(
                    out=maskv_sb[:], in0=tsum_sb[:], scalar1=0.0, scalar2=None,
                    op0=AluOpType.is_equal,
                )
                # stored with x_out at the end of L1(b)
                if not hasattr(nc, "_maskv_t"):
                    nc._maskv_t = {}
                nc._maskv_t[b] = maskv_sb

            IND_CHUNKS = ((0, 20), (20, 19))

            def emit_layer(b, layer):
                x_cur = x_t[b]
                if layer == 0 and b > 0:
                    # compute this batch's indicator on DVE
                    at_bc1 = at_t[b].rearrange("p (o l) -> p o l", o=1)
                    for r0, rn in IND_CHUNKS:
                        nc.vector.tensor_tensor(
                            out=ind_full[:, b, r0 : r0 + rn, :],
                            in0=at_bc1.to_broadcast([L, rn, L]),
                            in1=rpat_sb[:, r0 : r0 + rn, :],
                            op=AluOpType.is_equal,
                        )
                ind_sb = ind_t[b]

                # z[m, r, d] = x[m, d] * deprel[r+1, d]   (chunked for overlap;
                # a 4-relation slice goes to GPSIMD except on the first burst)
                z_sb = bpool.tile([L, NR, D], dt.bfloat16, tag="z")
                x_bc1 = x_cur[:].rearrange("p (o d) -> p o d", o=1)
                for r0, rn in Z_CHUNKS_FULL:
                    nc.vector.tensor_tensor(
                        out=z_sb[:, r0 : r0 + rn, :],
                        in0=x_bc1.to_broadcast([L, rn, D]),
                        in1=dep_sb[:, r0 : r0 + rn, :],
                        op=AluOpType.mult,
                    )

                # Ax + x accumulated in PSUM (r in DMA-arrival order)
                acc_ps = psA.tile([L, D], dt.float32, tag="psA")
                for ri, r in enumerate(R_ORDER):
                    nc.tensor.matmul(
                        out=acc_ps[:],
                        lhsT=ind_sb[:, r, :],
                        rhs=z_sb[:, r, :],
                        start=(ri == 0),
                        stop=False,
                    )
                nc.tensor.matmul(
                    out=acc_ps[:], lhsT=ident_sb, rhs=x_cur[:], start=False, stop=True
                )

                # transpose (Ax+x) to feed the W matmul
                axx_sb = wpool.tile([L, D], dt.bfloat16)
                nc.scalar.activation(out=axx_sb[:], in_=acc_ps[:], func=Copy)
                xt_sb = wpool.tile([L, D], dt.bfloat16)
                for h in range(2):
                    xt_ps = psC.tile([L, L], dt.bfloat16, tag="psC")
                    nc.tensor.transpose(
                        out=xt_ps[:],
                        in_=axx_sb[:, h * L : (h + 1) * L],
                        identity=ident_sb,
                    )
                    nc.scalar.activation(
                        out=xt_sb[:, h * L : (h + 1) * L], in_=xt_ps[:], func=Copy
                    )

                # (Ax+x) @ W.T + 2b
                out2_ps = psB.tile([L, D], dt.float32, tag="psB")
                nc.tensor.matmul(
                    out=out2_ps[:], lhsT=xt_sb[:, 0:L],
                    rhs=w_sb[:, layer * 400 : layer * 400 + D],
                    start=True, stop=False,
                )
                nc.tensor.matmul(
                    out=out2_ps[:], lhsT=xt_sb[:, L : 2 * L],
                    rhs=w_sb[:, layer * 400 + D : layer * 400 + 2 * D],
                    start=False, stop=False,
                )
                nc.tensor.matmul(
                    out=out2_ps[:], lhsT=onesr_sb,
                    rhs=w2b_sb[:, layer * D : (layer + 1) * D],
                    start=False, stop=True,
                )

                # relu(out2 / denom)
                if layer == 0:
                    x_nxt = wpool.tile([L, D], dt.bfloat16, tag="x", bufs=BLOC)
                    nc.scalar.activation(
                        out=x_nxt[:], in_=out2_ps[:], func=Relu,
                        scale=recip_t[b][:, 0:1],
                    )
                    x_t[b] = x_nxt
                else:
                    xm_sb = wpool.tile([L, D + 1], dt.float32)
                    nc.scalar.activation(
                        out=xm_sb[:, 0:D], in_=out2_ps[:], func=Relu,
                        scale=recip_t[b][:, 0:1],
                    )
                    nc.vector.tensor_copy(out=xm_sb[:, D : D + 1], in_=nc._maskv_t[b][:])
                    nc.sync.dma_start(out=xm_out[b], in_=xm_sb[:])

            # b-major interleave: L0(b0) L0(b1) L1(b0) L0(b2) L1(b1) ...
            order = [(0, 0), (1, 0), (0, 1), (2, 0), (1, 1), (3, 0), (2, 1), (3, 1)]
            for b, layer in order:
                emit_layer(b, layer)

    nc.compile()
    return nc


def _get_program():
    global _PROGRAM
    if _PROGRAM is None:
        _PROGRAM = _build_program()
    return _PROGRAM


def kernel(adj, words, pos, ner, emb, pos_emb, ner_emb, deprel_emb,
           Wp, bp, W0, b0, W1, b1):
    from concourse.bass_utils import run_bass_kernel_spmd

    adj = np.asarray(adj)
    words = np.asarray(words)
    pos = np.asarray(pos)
    ner = np.asarray(ner)
    emb = np.asarray(emb, dtype=np.float32)
    pos_emb = np.asarray(pos_emb, dtype=np.float32)
    ner_emb = np.asarray(ner_emb, dtype=np.float32)
    deprel_emb = np.asarray(deprel_emb, dtype=np.float32)
    Wp = np.asarray(Wp, dtype=np.float32)
    bp = np.asarray(bp, dtype=np.float32)
    W0 = np.asarray(W0, dtype=np.float32)
    b0 = np.asarray(b0, dtype=np.float32)
    W1 = np.asarray(W1, dtype=np.float32)
    b1 = np.asarray(b1, dtype=np.float32)

    # ---- constant folding on host (weights only) ----
    wtab = (emb @ Wp[:, :300].T).astype(BF16)  # [50000, 200]
    pos_proj = pos_emb @ Wp[:, 300:330].T  # [50, 200]
    ner_proj = ner_emb @ Wp[:, 330:360].T  # [10, 200]
    ptab = (pos_proj[:, None, :] + ner_proj[None, :, :] + bp).reshape(500, D)
    gtab = np.concatenate([wtab, ptab.astype(BF16)], axis=0)  # [50500, 200]

    cbuf = np.zeros((L, CW), dtype=np.float32)
    for li, (W, bias) in enumerate(((W0, b0), (W1, b1))):
        WT = W.T
        cbuf[:, OFF_WAUG + li * 400 : OFF_WAUG + li * 400 + D] = WT[0:L, :]
        cbuf[:, OFF_WAUG + li * 400 + D : OFF_WAUG + li * 400 + 2 * D] = WT[L : 2 * L, :]
    cbuf[:, OFF_IDENT:OFF_ONESC] = np.eye(L, dtype=np.float32)
    cbuf[:, OFF_ONESC] = 1.0
    cbuf = cbuf.astype(BF16)
    depb = np.broadcast_to(
        deprel_emb[1:R].reshape(1, NR * D), (L, NR * D)
    ).astype(BF16)
    rpatb = np.broadcast_to(
        np.repeat(np.arange(1, R, dtype=np.float32), L).reshape(1, NR * L), (L, NR * L)
    ).astype(BF16)

    consts1 = np.zeros((1, 2 * D + L), dtype=np.float32)
    consts1[0, 0:D] = 2.0 * b0
    consts1[0, D : 2 * D] = 2.0 * b1
    consts1[0, 2 * D :] = 1.0
    consts1 = consts1.astype(BF16)

    pn_idx = (pos.astype(np.int64) * 10 + ner.astype(np.int64)).astype(np.int32)
    gidx_full = np.stack(
        [words.astype(np.int32), pn_idx + 50000], axis=-1
    )  # [B, L, 2]

    nc = _get_program()
    in_maps = []
    for c in range(NCORES):
        s = slice(c * BLOC, (c + 1) * BLOC)
        # adjTall[m, b*L + l] = adj[cb+b, l, m]; gall[l, b*2+j]
        adjT_3 = adj[s].transpose(2, 0, 1)  # [m, b, l]
        adjT_c = np.ascontiguousarray(adjT_3.reshape(L, BLOC * L)).astype(BF16)
        # b0's one-hot indicator: ind0b[m, r, l] = (adj[cb, l, m] == r+1)
        ind0_c = (
            adjT_3[:, 0, None, :] == np.arange(1, R, dtype=adjT_3.dtype)[None, :, None]
        ).reshape(L, NR * L).astype(BF16)
        gall_c = np.ascontiguousarray(
            gidx_full[s].transpose(1, 0, 2).reshape(L, BLOC * 2)
        )
        in_maps.append(
            {
                "adjTall": adjT_c,
                "gall": gall_c,
                "gtab": gtab,
                "cb": cbuf,
                "ind0b": ind0_c,
                "rpatb": rpatb,
                "depb": depb,
                "consts1": consts1,
            }
        )

    res = run_bass_kernel_spmd(nc, in_maps, list(range(NCORES)))

    xm = np.concatenate([res.results[c]["xm_out"] for c in range(NCORES)], axis=0)
    x_full = np.ascontiguousarray(xm[:, :, 0:D]).astype(np.float32)
    mask_full = xm[:, :, D : D + 1] != 0.0
    return x_full, mask_full


# revision 45
# speedup vs baseline: 1.0901x; 1.0901x over previous
"""GCN message-passing kernel for Trainium2, 8-core data-parallel.

Reference computation (B=32, L=100, D=200, R=40 relations):
    embs = concat(emb[words], pos_emb[pos], ner_emb[ner])   [B,L,360]
    x = embs @ Wp.T + bp                                    [B,L,200]
    for (W,b) in layers:
        Ax[b,l,d] = sum_m deprel_emb[adj[b,l,m], d] * x[b,m,d]
        x = relu(((Ax + x) @ W.T + 2b) / denom)
    returns (x, mask)

Key identity used on device (exact):
    Ax = sum_{r=1..39} Ind_r @ (x * deprel[r])     Ind_r[m,l] = (adj[l,m]==r)
so the per-edge [B,L,L,D] gather never materializes; instead 39 masked
matmuls accumulate in PSUM per (batch, layer).

Sharding: batch axis across 8 cores (4 items each). Embedding tables are
constant-folded on the host (emb @ Wp_words.T; pos/ner projected into one
500-row table) and gathered on-device by index via indirect DMA.
"""

import sys

sys.path.insert(0, "/opt/trn_rl_repo")

import numpy as np
import ml_dtypes

BF16 = ml_dtypes.bfloat16

B, L, D = 32, 100, 200
R = 40  # deprel vocab (row 0 is padding -> dropped)
NR = R - 1  # 39 real relations
NCORES = 8
BLOC = B // NCORES  # 4 batch items per core

# "cb" consts layout (bf16, [100, CW]): waug | ident | onesc
OFF_WAUG = 0
OFF_IDENT = OFF_WAUG + 800        # 800
OFF_ONESC = OFF_IDENT + L         # 900
CW = OFF_ONESC + 1                # 901

_PROGRAM = None


def _build_program():
    import concourse.bass as bass
    import concourse.bacc as bacc
    import concourse.mybir as mybir
    from concourse.tile import TileContext
    from concourse.alu_op_type import AluOpType

    dt = mybir.dt
    nc = bacc.Bacc()

    # ---- DRAM parameters (per-core inputs) ----
    # adjTall: [L, BLOC*L] = all batches' transposed adj side by side
    # gall:    [L, BLOC*2] = all batches' gather indices side by side
    adjTall = nc.declare_dram_parameter(
        "adjTall", [L, BLOC * L], dt.bfloat16, isOutput=False
    )
    gall = nc.declare_dram_parameter("gall", [L, BLOC * 2], dt.int32, isOutput=False)
    gtab = nc.declare_dram_parameter("gtab", [50500, D], dt.bfloat16, isOutput=False)
    cb = nc.declare_dram_parameter("cb", [L, CW], dt.bfloat16, isOutput=False)
    rpatb = nc.declare_dram_parameter("rpatb", [L, NR * L], dt.bfloat16, isOutput=False)
    depb = nc.declare_dram_parameter("depb", [L, NR * D], dt.bfloat16, isOutput=False)
    consts1 = nc.declare_dram_parameter("consts1", [1, 2 * D + L], dt.bfloat16, isOutput=False)

    xm_out = nc.declare_dram_parameter(
        "xm_out", [BLOC, L, D + 1], dt.float32, isOutput=True
    )

    AX = mybir.AxisListType.X
    Relu = mybir.ActivationFunctionType.Relu
    Copy = mybir.ActivationFunctionType.Copy

    # arrival-ordered: dep r 26..38 lands first, then 0..12, then 13..25
    Z_CHUNKS_FULL = ((26, 13), (0, 13), (13, 13))
    R_ORDER = list(range(26, 39)) + list(range(0, 26))
    IND_CHUNKS = ((0, 20), (20, 19))

    with TileContext(nc) as tc:
        with (
            tc.tile_pool(name="consts", bufs=1) as cpool,
            tc.tile_pool(name="work", bufs=2) as wpool,
            tc.tile_pool(name="big", bufs=3) as bpool,
            tc.tile_pool(name="psA", bufs=2, space="PSUM") as psA,
            tc.tile_pool(name="psB", bufs=2, space="PSUM") as psB,
            tc.tile_pool(name="psC", bufs=2, space="PSUM") as psC,
        ):
            # ---- dual-queue load schedule, priority-ordered: tiny/hot loads
            # fully land before the fat deprel stream enters the rings ----
            ga_sb = wpool.tile([L, BLOC * 2], dt.int32, bufs=1)
            nc.sync.dma_start(out=ga_sb[:], in_=gall[:])
            atall_sb = wpool.tile([L, BLOC * L], dt.bfloat16, bufs=1)
            nc.scalar.dma_start(out=atall_sb[:], in_=adjTall[:])
            gi_t = [ga_sb[:, 2 * b : 2 * b + 2] for b in range(BLOC)]
            at_t = [atall_sb[:, b * L : (b + 1) * L] for b in range(BLOC)]

            c_sb = cpool.tile([L, CW], dt.bfloat16)
            nc.scalar.dma_start(out=c_sb[:], in_=cb[:])
            c1_sb = cpool.tile([1, 2 * D + L], dt.bfloat16)
            nc.scalar.dma_start(out=c1_sb[:], in_=consts1[:])

            rpat_sb = cpool.tile([L, NR, L], dt.bfloat16)
            nc.sync.dma_start(
                out=rpat_sb[:], in_=rpatb[:].rearrange("p (r l) -> p r l", r=NR)
            )

            dep_sb = cpool.tile([L, NR, D], dt.bfloat16)
            depb_v = depb[:].rearrange("p (r d) -> p r d", r=NR)
            # fat stream last; r 26..38 first (einsum consumes in this order)
            nc.scalar.dma_start(out=dep_sb[:, 26:39, :], in_=depb_v[:, 26:39, :])
            nc.sync.dma_start(out=dep_sb[:, 0:13, :], in_=depb_v[:, 0:13, :])
            nc.scalar.dma_start(out=dep_sb[:, 13:26, :], in_=depb_v[:, 13:26, :])

            w_sb = c_sb[:, OFF_WAUG:OFF_IDENT]
            ident_sb = c_sb[:, OFF_IDENT:OFF_ONESC]
            onesc_sb = c_sb[:, OFF_ONESC : OFF_ONESC + 1]
            w2b_sb = c1_sb[:, 0 : 2 * D]
            onesr_sb = c1_sb[:, 2 * D : 2 * D + L]

            # ---- phase A per b: gathers, x0, denom/mask ----
            x_t = [None] * BLOC
            recip_t = [None] * BLOC
            ind_t = [None] * BLOC
            for b in range(BLOC):
                wgpn_sb = wpool.tile([L, 2, D], dt.bfloat16)
                for j in range(2):
                    nc.gpsimd.indirect_dma_start(
                        out=wgpn_sb[:, j, :],
                        out_offset=None,
                        in_=gtab[:],
                        in_offset=bass.IndirectOffsetOnAxis(
                            ap=gi_t[b][:, j : j + 1], axis=0
                        ),
                    )
                x0_sb = wpool.tile([L, D], dt.bfloat16, tag="x", bufs=BLOC)
                nc.gpsimd.tensor_tensor(
                    out=x0_sb[:], in0=wgpn_sb[:, 0, :], in1=wgpn_sb[:, 1, :],
                    op=AluOpType.add,
                )
                x_t[b] = x0_sb

                # nz = min(adjT, 1): exact nonzero indicator for values 0..39
                nz_sb = wpool.tile([L, L], dt.bfloat16)
                nc.vector.tensor_scalar(
                    out=nz_sb[:], in0=at_t[b], scalar1=1.0, scalar2=None,
                    op0=AluOpType.min,
                )
                dn_ps = psC.tile([L, 1], dt.float32, tag="psC")
                nc.tensor.matmul(
                    out=dn_ps[:], lhsT=nz_sb[:], rhs=onesc_sb, start=True, stop=True
                )
                dnp1_sb = wpool.tile([L, 1], dt.float32)
                nc.scalar.add(out=dnp1_sb[:], in_=dn_ps[:], add=1.0)
                recip_sb = wpool.tile([L, 1], dt.float32, bufs=BLOC)
                nc.vector.reciprocal(out=recip_sb[:], in_=dnp1_sb[:])
                recip_t[b] = recip_sb

                msum_sb = wpool.tile([L, 1], dt.float32)
                nc.vector.tensor_reduce(
                    out=msum_sb[:], in_=nz_sb[:], axis=AX, op=AluOpType.add
                )
                tsum_sb = wpool.tile([L, 1], dt.float32)
                nc.vector.tensor_tensor(
                    out=tsum_sb[:], in0=msum_sb[:], in1=dn_ps[:], op=AluOpType.add
                )
                maskv_sb = wpool.tile([L, 1], dt.float32, bufs=BLOC)
                nc.vector.tensor_scalar(
                    out=maskv_sb[:], in0=tsum_sb[:], scalar1=0.0, scalar2=None,
                    op0=AluOpType.is_equal,
                )
                # stored with x_out at the end of L1(b)
                if not hasattr(nc, "_maskv_t"):
                    nc._maskv_t = {}
                nc._maskv_t[b] = maskv_sb

            def emit_layer(b, layer):
                x_cur = x_t[b]

                if layer == 0:
                    # indicator first: only needs adjT + rpat (ready early)
                    ind_sb = bpool.tile([L, NR, L], dt.bfloat16, tag="ind", bufs=BLOC)
                    at_bc1 = at_t[b].rearrange("p (o l) -> p o l", o=1)
                    for r0, rn in IND_CHUNKS:
                        nc.vector.tensor_tensor(
                            out=ind_sb[:, r0 : r0 + rn, :],
                            in0=at_bc1.to_broadcast([L, rn, L]),
                            in1=rpat_sb[:, r0 : r0 + rn, :],
                            op=AluOpType.is_equal,
                        )
                    ind_t[b] = ind_sb
                ind_sb = ind_t[b]

                # z[m, r, d] = x[m, d] * deprel[r+1, d]   (chunked for overlap;
                # a 4-relation slice goes to GPSIMD except on the first burst)
                z_sb = bpool.tile([L, NR, D], dt.bfloat16, tag="z")
                x_bc1 = x_cur[:].rearrange("p (o d) -> p o d", o=1)
                for r0, rn in Z_CHUNKS_FULL:
                    nc.vector.tensor_tensor(
                        out=z_sb[:, r0 : r0 + rn, :],
                        in0=x_bc1.to_broadcast([L, rn, D]),
                        in1=dep_sb[:, r0 : r0 + rn, :],
                        op=AluOpType.mult,
                    )

                # Ax + x accumulated in PSUM (r in DMA-arrival order)
                acc_ps = psA.tile([L, D], dt.float32, tag="psA")
                for ri, r in enumerate(R_ORDER):
                    nc.tensor.matmul(
                        out=acc_ps[:],
                        lhsT=ind_sb[:, r, :],
                        rhs=z_sb[:, r, :],
                        start=(ri == 0),
                        stop=False,
                    )
                nc.tensor.matmul(
                    out=acc_ps[:], lhsT=ident_sb, rhs=x_cur[:], start=False, stop=True
                )

                # transpose (Ax+x) to feed the W matmul
                axx_sb = wpool.tile([L, D], dt.bfloat16)
                nc.scalar.activation(out=axx_sb[:], in_=acc_ps[:], func=Copy)
                xt_sb = wpool.tile([L, D], dt.bfloat16)
                for h in range(2):
                    xt_ps = psC.tile([L, L], dt.bfloat16, tag="psC")
                    nc.tensor.transpose(
                        out=xt_ps[:],
                        in_=axx_sb[:, h * L : (h + 1) * L],
                        identity=ident_sb,
                    )
                    nc.scalar.activation(
                        out=xt_sb[:, h * L : (h + 1) * L], in_=xt_ps[:], func=Copy
                    )

                # (Ax+x) @ W.T + 2b
                out2_ps = psB.tile([L, D], dt.float32, tag="psB")
                nc.tensor.matmul(
                    out=out2_ps[:], lhsT=xt_sb[:, 0:L],
                    rhs=w_sb[:, layer * 400 : layer * 400 + D],
                    start=True, stop=False,
                )
                nc.tensor.matmul(
                    out=out2_ps[:], lhsT=xt_sb[:, L : 2 * L],
                    rhs=w_sb[:, layer * 400 + D : layer * 400 + 2 * D],
                    start=False, stop=False,
                )
                nc.tensor.matmul(
                    out=out2_ps[:], lhsT=onesr_sb,
                    rhs=w2b_sb[:, layer * D : (layer + 1) * D],
                    start=False, stop=True,
                )

                # relu(out2 / denom)
                if layer == 0:
                    x_nxt = wpool.tile([L, D], dt.bfloat16, tag="x", bufs=BLOC)
                    nc.scalar.activation(
                        out=x_nxt[:], in_=out2_ps[:], func=Relu,
                        scale=recip_t[b][:, 0:1],
                    )
                    x_t[b] = x_nxt
                else:
                    xm_sb = wpool.tile([L, D + 1], dt.float32)
                    nc.scalar.activation(
                        out=xm_sb[:, 0:D], in_=out2_ps[:], func=Relu,
                        scale=recip_t[b][:, 0:1],
                    )
                    nc.vector.tensor_copy(out=xm_sb[:, D : D + 1], in_=nc._maskv_t[b][:])
                    nc.sync.dma_start(out=xm_out[b], in_=xm_sb[:])

            # b-major interleave: L0(b0) L0(b1) L1(b0) L0(b2) L1(b1) ...
            order = [(0, 0), (1, 0), (0, 1), (2, 0), (1, 1), (3, 0), (2, 1), (3, 1)]
            for b, layer in order:
                emit_layer(b, layer)

    nc.compile()
    return nc


def _get_program():
    global _PROGRAM
    if _PROGRAM is None:
        _PROGRAM = _build_program()
    return _PROGRAM


def kernel(adj, words, pos, ner, emb, pos_emb, ner_emb, deprel_emb,
           Wp, bp, W0, b0, W1, b1):
    from concourse.bass_utils import run_bass_kernel_spmd

    adj = np.asarray(adj)
    words = np.asarray(words)
    pos = np.asarray(pos)
    ner = np.asarray(ner)
    emb = np.asarray(emb, dtype=np.float32)
    pos_emb = np.asarray(pos_emb, dtype=np.float32)
    ner_emb = np.asarray(ner_emb, dtype=np.float32)
    deprel_emb = np.asarray(deprel_emb, dtype=np.float32)
    Wp = np.asarray(Wp, dtype=np.float32)
    bp = np.asarray(bp, dtype=np.float32)
    W0 = np.asarray(W0, dtype=np.float32)
    b0 = np.asarray(b0, dtype=np.float32)
    W1 = np.asarray(W1, dtype=np.float32)
    b1 = np.asarray(b1, dtype=np.float32)

    # ---- constant folding on host (weights only) ----
    wtab = (emb @ Wp[:, :300].T).astype(BF16)  # [50000, 200]
    pos_proj = pos_emb @ Wp[:, 300:330].T  # [50, 200]
    ner_proj = ner_emb @ Wp[:, 330:360].T  # [10, 200]
    ptab = (pos_proj[:, None, :] + ner_proj[None, :, :] + bp).reshape(500, D)
    gtab = np.concatenate([wtab, ptab.astype(BF16)], axis=0)  # [50500, 200]

    cbuf = np.zeros((L, CW), dtype=np.float32)
    for li, (W, bias) in enumerate(((W0, b0), (W1, b1))):
        WT = W.T
        cbuf[:, OFF_WAUG + li * 400 : OFF_WAUG + li * 400 + D] = WT[0:L, :]
        cbuf[:, OFF_WAUG + li * 400 + D : OFF_WAUG + li * 400 + 2 * D] = WT[L : 2 * L, :]
    cbuf[:, OFF_IDENT:OFF_ONESC] = np.eye(L, dtype=np.float32)
    cbuf[:, OFF_ONESC] = 1.0
    cbuf = cbuf.astype(BF16)
    rpatb = np.broadcast_to(
        np.repeat(np.arange(1, R, dtype=np.float32), L).reshape(1, NR * L), (L, NR * L)
    ).astype(BF16)
    depb = np.broadcast_to(
        deprel_emb[1:R].reshape(1, NR * D), (L, NR * D)
    ).astype(BF16)

    consts1 = np.zeros((1, 2 * D + L), dtype=np.float32)
    consts1[0, 0:D] = 2.0 * b0
    consts1[0, D : 2 * D] = 2.0 * b1
    consts1[0, 2 * D :] = 1.0
    consts1 = consts1.astype(BF16)

    pn_idx = (pos.astype(np.int64) * 10 + ner.astype(np.int64)).astype(np.int32)
    gidx_full = np.stack(
        [words.astype(np.int32), pn_idx + 50000], axis=-1
    )  # [B, L, 2]

    nc = _get_program()
    in_maps = []
    for c in range(NCORES):
        s = slice(c * BLOC, (c + 1) * BLOC)
        # adjTall[m, b*L + l] = adj[cb+b, l, m]; gall[l, b*2+j]
        adjT_c = np.ascontiguousarray(
            adj[s].transpose(2, 0, 1).reshape(L, BLOC * L)
        ).astype(BF16)
        gall_c = np.ascontiguousarray(
            gidx_full[s].transpose(1, 0, 2).reshape(L, BLOC * 2)
        )
        in_maps.append(
            {
                "adjTall": adjT_c,
                "gall": gall_c,
                "gtab": gtab,
                "cb": cbuf,
                "rpatb": rpatb,
                "depb": depb,
                "consts1": consts1,
            }
        )

    res = run_bass_kernel_spmd(nc, in_maps, list(range(NCORES)))

    xm = np.concatenate([res.results[c]["xm_out"] for c in range(NCORES)], axis=0)
    x_full = np.ascontiguousarray(xm[:, :, 0:D]).astype(np.float32)
    mask_full = xm[:, :, D : D + 1] != 0.0
    return x_full, mask_full
